# revision 3
# baseline (speedup 1.0000x reference)
"""Trainium2 Bass kernel for nn_Encoder (voxel scatter-mean encoder).

Computation (per batch sample b):
    vox   = trunc(points / 0.1)
    key   = voxel hash of vox (injective)
    avg   = per-voxel mean of feats, gathered back per point
    dist  = || points/0.1 - (vox + 0.05) ||_2
    out   = concat([feats, avg * dist + feats], axis=-1)

Sharding: batch dim (2 samples) x voxel-key range partition (4 ways) = 8 cores.
The host groups each sample's points by voxel key and packs whole segments
(voxel groups) into 128-point tiles, so every voxel's points live in exactly
one 128-row tile on one core.  The device kernel then computes, per tile:

    E      = equality matrix  E[i,j] = (key_i == key_j)     (DVE, vs PE-transposed keys)
    P      = E @ F            per-point gathered segment sums (PE matmul, E symmetric)
    out_a  = P * s + F        with s = dist/cnt precomputed per point (DVE fused op)
    out    = [F, out_a]       (GPSIMD copies F half), one contiguous DMA store

Segments larger than 128 points (the double-width origin voxel) are split for
device processing and their rows are patched exactly on the host afterwards.
"""

import math
import os
from contextlib import ExitStack

import numpy as np

# ---------------------------------------------------------------- constants
UNIT = np.float32(0.1)
HALF = np.float32(0.05)
P = 128          # points per tile == partitions
C = 128          # feature channels
TPC = 16         # tiles per DMA chunk (1 MiB loads / 2 MiB stores)
N_CORES = 8
SHARDS_PER_SAMPLE = 4
PAD_KEY = np.float32(2.5e7)   # exact in fp32, larger than any real segment id

_compiled_cache: dict = {}


# ---------------------------------------------------------------- host prep
def _plan_sample(pts: np.ndarray, feats: np.ndarray):
    """Group one sample's points by voxel key and lay them out for the device.

    Returns (shards, patches) where shards is a list of dicts with
    per-shard device arrays/indices and patches holds oversized segments
    that the host fixes up exactly after the device run.
    """
    n = pts.shape[0]
    q = pts / UNIT                      # fp32, same rounding as reference
    vox = np.trunc(q)
    d = q - (vox + HALF)
    dist = np.sqrt((d * d).sum(axis=1, dtype=np.float32)).astype(np.float32)

    iv = vox.astype(np.int64)
    lo = iv.min(axis=0)
    span = iv.max(axis=0) - lo + 1
    key = ((iv[:, 0] - lo[0]) * span[1] + (iv[:, 1] - lo[1])) * span[2] + (
        iv[:, 2] - lo[2]
    )

    order = np.argsort(key)
    sk = key[order]
    newseg = np.empty(n, dtype=bool)
    newseg[0] = True
    np.not_equal(sk[1:], sk[:-1], out=newseg[1:])
    seg_first = np.flatnonzero(newseg)
    seg_sizes = np.diff(np.append(seg_first, n))

    # oversized segments: split for the device, exact host patch afterwards
    patches = []
    for f0, sz in zip(seg_first[seg_sizes > P], seg_sizes[seg_sizes > P]):
        patches.append(order[f0 : f0 + sz])

    nsub = (seg_sizes + P - 1) // P
    nsub_total = int(nsub.sum())
    seg_of_sub = np.repeat(np.arange(len(seg_first)), nsub)
    sub_ord = np.arange(nsub_total) - np.repeat(
        np.concatenate(([0], np.cumsum(nsub)[:-1])), nsub
    )
    sub_start = seg_first[seg_of_sub] + sub_ord * P
    sub_size = np.minimum(seg_sizes[seg_of_sub] - sub_ord * P, P).astype(np.int64)

    # balanced contiguous key-range partition into 4 shards (by point count)
    cum = np.cumsum(sub_size)
    shard_of_sub = np.minimum(
        (cum - 1) * SHARDS_PER_SAMPLE // n, SHARDS_PER_SAMPLE - 1
    )

    shards = []
    for s in range(SHARDS_PER_SAMPLE):
        m = shard_of_sub == s
        starts = sub_start[m]
        sizes = sub_size[m]
        # next-fit pack whole sub-segments into 128-slot tiles
        sizes_l = sizes.tolist()
        offs = np.empty(len(sizes_l), dtype=np.int64)
        fill = 0
        tile_i = 0
        for i, sz in enumerate(sizes_l):
            if fill + sz > P:
                tile_i += 1
                fill = 0
            offs[i] = tile_i * P + fill
            fill += sz
        ntiles = tile_i + 1 if sizes_l else 1

        total = int(sizes.sum())
        excl = np.concatenate(([0], np.cumsum(sizes)[:-1]))
        within = np.arange(total) - np.repeat(excl, sizes)
        sorted_pos = np.repeat(starts, sizes) + within
        orig = order[sorted_pos]
        devpos = np.repeat(offs, sizes) + within
        kval = np.repeat(np.arange(len(sizes_l), dtype=np.float32), sizes)
        sval = dist[orig] / np.repeat(sizes.astype(np.float32), sizes)

        shards.append(
            dict(
                ntiles=ntiles,
                orig=orig,
                devpos=devpos,
                kval=kval,
                sval=sval,
            )
        )
    return shards, patches


def _build_device_inputs(shards_flat, feats_by_shard, ntiles):
    """Pad all shards to a common tile count and build device-layout arrays."""
    chunks = ntiles // TPC
    ns = ntiles * P
    in_maps = []
    for sh, feats in zip(shards_flat, feats_by_shard):
        f_flat = np.zeros((ns, C), dtype=np.float32)
        k_flat = np.full(ns, PAD_KEY, dtype=np.float32)
        s_flat = np.zeros(ns, dtype=np.float32)
        dp = sh["devpos"]
        f_flat[dp] = feats[sh["orig"]]
        k_flat[dp] = sh["kval"]
        s_flat[dp] = sh["sval"]
        # device layout: f_in[c, p, t*C:(t+1)*C] = feats of point c*TPC*P + t*P + p
        f_dev = np.ascontiguousarray(
            f_flat.reshape(chunks, TPC, P, C).transpose(0, 2, 1, 3)
        ).reshape(chunks, P, TPC * C)
        k_t = np.ascontiguousarray(k_flat.reshape(ntiles, P).T)
        s_t = np.ascontiguousarray(s_flat.reshape(ntiles, P).T)
        in_maps.append({"f_in": f_dev, "k_t": k_t, "s_t": s_t})
    return in_maps


# ---------------------------------------------------------------- device code
def _build_program(ntiles):
    import concourse.bass as bass
    import concourse.mybir as mybir
    import concourse.tile as tile
    from concourse import bacc
    from concourse.masks import make_identity

    f32 = mybir.dt.float32
    chunks = ntiles // TPC

    nc = bacc.Bacc(
        "TRN2",
        target_bir_lowering=False,
        debug=False,
        enable_asserts=False,
        num_devices=N_CORES,
    )
    f_in = nc.dram_tensor("f_in", (chunks, P, TPC * C), f32, kind="ExternalInput").ap()
    k_t = nc.dram_tensor("k_t", (P, ntiles), f32, kind="ExternalInput").ap()
    s_t = nc.dram_tensor("s_t", (P, ntiles), f32, kind="ExternalInput").ap()
    out = nc.dram_tensor(
        "out", (chunks, P, TPC * 2 * C), f32, kind="ExternalOutput"
    ).ap()

    grp = 4  # tiles per batched equality op (one PSUM bank = 4 * 128 fp32)

    with tile.TileContext(nc) as tc, ExitStack() as ctx:
        const = ctx.enter_context(tc.tile_pool(name="const", bufs=1))
        fpool = ctx.enter_context(tc.tile_pool(name="f", bufs=3))
        apool = ctx.enter_context(tc.tile_pool(name="a", bufs=3))
        epool = ctx.enter_context(tc.tile_pool(name="e", bufs=4))
        pa = ctx.enter_context(tc.tile_pool(name="pa", bufs=2, space="PSUM"))
        pb = ctx.enter_context(tc.tile_pool(name="pb", bufs=4, space="PSUM"))

        ident = const.tile([P, P], f32)
        make_identity(nc, ident[:])
        kt_sb = const.tile([P, ntiles], f32)
        nc.sync.dma_start(kt_sb[:], k_t[:])
        st_sb = const.tile([P, ntiles], f32)
        nc.sync.dma_start(st_sb[:], s_t[:])

        for ci in range(chunks):
            f = fpool.tile([P, TPC * C], f32)
            nc.sync.dma_start(f[:], f_in[ci])
            abuf = apool.tile([P, TPC * C], f32)
            for g in range(TPC // grp):
                t0 = g * grp
                ti0 = ci * TPC + t0
                psa = pa.tile([P, grp * P], f32)
                for j in range(grp):
                    kcol = kt_sb[:, ti0 + j : ti0 + j + 1]
                    nc.tensor.transpose(
                        psa[:, j * P : (j + 1) * P],
                        kcol.to_broadcast([P, P]),
                        ident[:],
                    )
                e4 = epool.tile([P, grp * P], f32)
                nc.vector.tensor_tensor(
                    e4[:].rearrange("p (t j) -> p t j", t=grp),
                    kt_sb[:, ti0 : ti0 + grp].to_broadcast([P, grp, P]),
                    psa[:].rearrange("p (t j) -> p t j", t=grp),
                    op=mybir.AluOpType.is_equal,
                )
                for j in range(grp):
                    t = t0 + j
                    ftile = f[:, t * C : (t + 1) * C]
                    psb = pb.tile([P, P], f32)
                    nc.tensor.matmul(
                        psb[:], lhsT=e4[:, j * P : (j + 1) * P], rhs=ftile,
                        start=True, stop=True,
                    )
                    nc.scalar.activation(
                        abuf[:, t * C : (t + 1) * C],
                        psb[:],
                        mybir.ActivationFunctionType.Copy,
                        scale=st_sb[:, ci * TPC + t : ci * TPC + t + 1],
                    )
            nc.vector.tensor_tensor(
                abuf[:], abuf[:], f[:], op=mybir.AluOpType.add
            )
            nc.sync.dma_start(out[ci][:, 0 : TPC * C], f[:])
            nc.sync.dma_start(out[ci][:, TPC * C : 2 * TPC * C], abuf[:])

    nc.compile()
    return nc


# ---------------------------------------------------------------- entry point
def kernel(gs_points: np.ndarray, gs_feats: np.ndarray) -> np.ndarray:
    from concourse.bass_utils import run_bass_kernel_spmd

    gs_points = np.asarray(gs_points, dtype=np.float32)
    gs_feats = np.asarray(gs_feats, dtype=np.float32)
    b_sz, n, c = gs_feats.shape
    assert c == C

    shards_flat = []
    feats_by_shard = []
    patches_by_sample = []
    for b in range(b_sz):
        shards, patches = _plan_sample(gs_points[b], gs_feats[b])
        patches_by_sample.append(patches)
        for sh in shards:
            shards_flat.append(sh)
            feats_by_shard.append(gs_feats[b])

    ntiles = max(sh["ntiles"] for sh in shards_flat)
    ntiles = ((ntiles + TPC - 1) // TPC) * TPC
    in_maps = _build_device_inputs(shards_flat, feats_by_shard, ntiles)

    if ntiles not in _compiled_cache:
        _compiled_cache[ntiles] = _build_program(ntiles)
    nc = _compiled_cache[ntiles]

    trace = bool(os.environ.get("KERNEL_PROFILE"))
    res = run_bass_kernel_spmd(
        nc, in_maps, core_ids=list(range(N_CORES)), trace=trace
    )
    if trace:
        kernel.last_exec_time_ns = res.exec_time_ns
        kernel.last_profile = res

    chunks = ntiles // TPC
    out_full = np.empty((b_sz, n, 2 * C), dtype=np.float32)
    for i, sh in enumerate(shards_flat):
        b = i // SHARDS_PER_SAMPLE
        dev = res.results[i]["out"]
        # out[c, p, half, t, :] -> row of point c*TPC*P + t*P + p is
        # concat over half of [c, p, half, t, :]
        out_flat = (
            dev.reshape(chunks, P, 2, TPC, C)
            .transpose(0, 3, 1, 2, 4)
            .reshape(ntiles * P, 2 * C)
        )
        out_full[b, sh["orig"]] = out_flat[sh["devpos"]]

    # exact host patch for segments that were split across tiles
    for b in range(b_sz):
        for orig in patches_by_sample[b]:
            rows = gs_feats[b][orig]
            mean = rows.sum(axis=0, dtype=np.float32) / np.float32(len(orig))
            q = gs_points[b][orig] / UNIT
            vox = np.trunc(q)
            dd = q - (vox + HALF)
            dist = np.sqrt((dd * dd).sum(axis=1, dtype=np.float32)).astype(
                np.float32
            )
            out_full[b, orig, :C] = rows
            out_full[b, orig, C:] = mean[None, :] * dist[:, None] + rows

    return out_full
